# revision 26
# baseline (speedup 1.0000x reference)
"""AttentionDecoderGRU single-step kernel for 8 Trainium2 NeuronCores.

Strategy (batch=1 matrix-vector workload, HBM-bandwidth bound):
  - Shard every weight matrix across the 8 cores:
      W_attn/W_comb/W_ih/W_hh by output dim (each core computes a 128-slice),
      W_out/b_out by vocab dim (6656 padded rows per core).
  - Embedding lookup (one row of emb) happens on the host (it is pure input
    sharding: only the selected row is shipped).
  - All matvecs run on the TensorEngine with host-pre-transposed weights.
    W_comb/W_ih/W_hh/W_out are bf16 (halves HBM traffic; rel err ~1e-3,
    threshold 2e-2); the attention path stays fp32r/fp32.
  - Small inputs are packed into 4 tensors so the serial chain isn't
    gated on a dozen individual DMA fixed costs.
  - 3 small AllGather collectives stitch the cores together: x (combine
    output), h_new, and the per-core (max, sumexp) pairs for log_softmax.
  - DMA rings are split: the 13 MB/core W_out stream goes through nc.sync
    (SP HWDGE ring) and is fully SBUF-resident before use; everything
    latency-critical goes through nc.scalar (ACT ring); collectives
    through gpsimd.

Outputs per core: logp shard [13, 512], h_new shard [1, 128], attn_w [1, 64].
The host concatenates shards and trims vocab padding.
"""

import ml_dtypes
import numpy as np

import concourse.bacc as bacc
import concourse.mybir as mybir
import concourse.tile as tile
from concourse.bass_utils import run_bass_kernel_spmd

N_CORES = 8
H = 1024
L = 64
V = 50257
HCH = 8          # H / 128 chunks
VPC = 6656       # padded vocab rows per core (= VT * TN)
VT = 13
TN = 512
NEG_BIG = -1.0e30

F32 = mybir.dt.float32
F32R = mybir.dt.float32r
BF16 = mybir.dt.bfloat16
AF = mybir.ActivationFunctionType
AX = mybir.AxisListType
OP = mybir.AluOpType

BF = ml_dtypes.bfloat16

# pack layouts (columns)
PA_W = 17 + 16 * L                # f32r: colsr | wattnT
PB_OFF_EYE = 17
PB_OFF_BOUT = 33
PB_OFF_ENC = 545
PB_OFF_WC = PB_OFF_ENC + H
PB1_W = PB_OFF_WC + 8 * H        # bf16: colsb | encb | wcombT[c=0..7]
PB2_W = 8 * H                    # bf16: wcombT[c=8..15]
PC_W = 2 * HCH * 384              # bf16: wihT | whhT
# packD (f32, 16 partitions): rows | eye16 | ones_row | bout_row
PD_ROWS = 0
PD_EYE = 1984
PD_ONESR = PD_EYE + 16
PD_PADC = PD_ONESR + 16
PD_W = PD_PADC + 1

TRACE = False
LAST_EXEC_NS = None
LAST_TRACE_PATH = None

_NC_CACHE = {}


def _build_nc(with_bias=True):
    key = ("nc", with_bias)
    if key in _NC_CACHE:
        return _NC_CACHE[key]

    nc = bacc.Bacc("TRN2", target_bir_lowering=False, debug=False,
                   num_devices=N_CORES)

    # ---- I/O ----
    packA_dr = nc.dram_tensor("packA", [128, PA_W], F32R, kind="ExternalInput")
    packB_dr = nc.dram_tensor("packB", [128, PB1_W], BF16, kind="ExternalInput")
    packB2_dr = nc.dram_tensor("packB2", [128, PB2_W], BF16, kind="ExternalInput")
    packC_dr = nc.dram_tensor("packC", [128, PC_W], BF16, kind="ExternalInput")
    packD_dr = nc.dram_tensor("packD", [16, PD_W], F32, kind="ExternalInput")
    wout_dr = nc.dram_tensor("woutT", [VT, 128, HCH, TN], BF16,
                             kind="ExternalInput")

    logp_out = nc.dram_tensor("logp", [VT, TN], F32, kind="ExternalOutput")
    hnew_out = nc.dram_tensor("hnew", [1, 128], F32, kind="ExternalOutput")
    attn_out = nc.dram_tensor("attnw", [1, L], F32, kind="ExternalOutput")

    with tile.TileContext(nc) as tc:
        with (
            tc.tile_pool(name="sb", bufs=1) as sb,
            tc.tile_pool(name="wpool", bufs=VT) as wpool,
            tc.tile_pool(name="rowp", bufs=3) as rowp,
            tc.tile_pool(name="ps", bufs=4, space="PSUM") as ps,
            tc.tile_pool(name="psL", bufs=4, space="PSUM") as psL,
            tc.tile_pool(name="dram", bufs=1, space="DRAM") as dram,
        ):
            # ---- packed small loads (ACT ring), in dependency order ----
            pA = sb.tile([128, PA_W], F32R)
            dma_pA = nc.scalar.dma_start(pA[:], packA_dr[:])
            pD = sb.tile([16, PD_W], F32)
            dma_pD = nc.scalar.dma_start(pD[:], packD_dr[:])
            pB = sb.tile([128, PB1_W], BF16)
            dma_pB = nc.scalar.dma_start(pB[:], packB_dr[:])
            pC = sb.tile([128, PC_W], BF16)
            dma_pC = nc.scalar.dma_start(pC[:], packC_dr[:])
            pB2 = sb.tile([128, PB2_W], BF16)
            dma_pB2 = nc.scalar.dma_start(pB2[:], packB2_dr[:])

            colsr = pA[:, 0:17]
            wattn = pA[:, 17:PA_W].rearrange("p (c n) -> p c n", c=16)
            colsb = pB[:, 0:17]
            eyeb = pB[0:16, PB_OFF_EYE:PB_OFF_EYE + 16]
            boutb = pB[0:VT, PB_OFF_BOUT:PB_OFF_BOUT + TN]
            encb = pB[0:64, PB_OFF_ENC:PB_OFF_ENC + H]
            wcomb = pB[:, PB_OFF_WC:PB1_W].rearrange("p (c n) -> p c n", c=HCH)
            wcomb2 = pB2[:].rearrange("p (c n) -> p c n", c=HCH)
            wih = pC[:, 0:HCH * 384].rearrange("p (c n) -> p c n", c=HCH)
            whh = pC[:, HCH * 384:PC_W].rearrange("p (c n) -> p c n", c=HCH)
            rows = pD[0:1, PD_ROWS:PD_ROWS + 1984]
            eye = pD[0:16, PD_EYE:PD_EYE + 16]
            onesr = pD[0:1, PD_ONESR:PD_ONESR + 16]
            padc = pD[0:1, PD_PADC:PD_PADC + 1]

            # W_out shard: fully SBUF-resident, 13 tile DMAs (SP ring),
            # staged behind chain events so the serial DMA pipe always
            # serves the latency-critical chain first.
            wo_tiles = []
            wo_dmas = []
            for t in range(VT):
                wo = wpool.tile([128, HCH, TN], BF16, tag="wo")
                dma_wo = nc.sync.dma_start(wo[:], wout_dr[t])
                wo_dmas.append(dma_wo)
                if t < 5:
                    tile.add_dep_helper(dma_wo.ins, dma_pB2.ins, sync=True,
                                        reason="wout after critical packs")
                wo_tiles.append(wo)

            # ---- attention scores: s = [e,h] @ W_attn.T + b_attn ----
            ps_s = ps.tile([1, L], F32, tag="ps")
            for c in range(16):
                nc.tensor.matmul(ps_s[:], colsr[:, c:c + 1], wattn[:, c, :],
                                 start=(c == 0), stop=(c == 15))
            es = sb.tile([1, L], F32)
            if with_bias:
                s_sb = sb.tile([1, L], F32)
                nc.vector.tensor_tensor(s_sb[:], ps_s[:], rows[:, 0:L],
                                        op=OP.add)
                nc.scalar.activation(es[:], s_sb[:], AF.Exp)
            else:
                nc.scalar.activation(es[:], ps_s[:], AF.Exp)
            ssum = sb.tile([1, 1], F32)
            nc.vector.reduce_sum(ssum[:], es[:], axis=AX.X)
            rcp = sb.tile([1, 1], F32)
            nc.vector.reciprocal(rcp[:], ssum[:])
            attnw_sb = sb.tile([1, L], F32)
            nc.vector.tensor_scalar_mul(attnw_sb[:], es[:], rcp[:])

            # attn_w row -> column [64, 1]
            ps_awc = ps.tile([L, 1], F32, tag="ps")
            nc.tensor.matmul(ps_awc[:], attnw_sb[:], onesr[0:1, 0:1],
                             start=True, stop=True)
            awc_sb = sb.tile([L, 1], BF16)
            nc.scalar.copy(awc_sb[:], ps_awc[:])

            # applied = attn_w @ enc, directly as bf16 column chunks [128, 8]
            ps_app = ps.tile([128, HCH], F32, tag="ps")
            for c in range(HCH):
                nc.tensor.matmul(ps_app[:, c:c + 1],
                                 encb[:, c * 128:(c + 1) * 128], awc_sb[:],
                                 start=True, stop=True)
            appc_sb = sb.tile([128, HCH], BF16)
            nc.scalar.copy(appc_sb[:], ps_app[:])

            ps_gh = ps.tile([1, 384], F32, tag="ps")
            for c in range(HCH):
                nc.tensor.matmul(ps_gh[:], colsb[:, 8 + c:9 + c], whh[:, c, :],
                                 start=(c == 0), stop=(c == 7))
            # ---- combine (replicated): x = relu([e, applied] @ W_comb.T + b) ----
            ps_x1 = ps.tile([1, 512], F32, tag="ps")
            ps_x2 = ps.tile([1, 512], F32, tag="ps")
            for c in range(HCH):
                nc.tensor.matmul(ps_x1[:], colsb[:, c:c + 1],
                                 wcomb[:, c, 0:512], start=(c == 0), stop=False)
                nc.tensor.matmul(ps_x2[:], colsb[:, c:c + 1],
                                 wcomb[:, c, 512:1024], start=(c == 0), stop=False)
            for c in range(HCH):
                nc.tensor.matmul(ps_x1[:], appc_sb[:, c:c + 1],
                                 wcomb2[:, c, 0:512], start=False, stop=(c == 7))
                nc.tensor.matmul(ps_x2[:], appc_sb[:, c:c + 1],
                                 wcomb2[:, c, 512:1024], start=False, stop=(c == 7))
            x_sb = sb.tile([1, H], F32)
            if with_bias:
                xr = sb.tile([1, H], F32)
                nc.vector.tensor_tensor(xr[:, 0:512], ps_x1[:],
                                        rows[:, 192:704], op=OP.add)
                nc.vector.tensor_tensor(xr[:, 512:1024], ps_x2[:],
                                        rows[:, 704:1216], op=OP.add)
                nc.vector.tensor_scalar_max(x_sb[:], xr[:], 0.0)
            else:
                nc.vector.tensor_scalar_max(x_sb[:, 0:512], ps_x1[:], 0.0)
                nc.vector.tensor_scalar_max(x_sb[:, 512:1024], ps_x2[:], 0.0)
            # x row -> bf16 column chunks for the gx matvecs
            ps_xc = ps.tile([128, HCH], F32, tag="ps")
            for c in range(HCH):
                nc.tensor.matmul(ps_xc[:, c:c + 1],
                                 x_sb[0:1, c * 128:(c + 1) * 128],
                                 onesr[0:1, 0:1], start=True, stop=True)
            xc_sb = sb.tile([128, HCH], BF16)
            nc.scalar.copy(xc_sb[:], ps_xc[:])

            # ---- GRU gates ----
            ps_gx = ps.tile([1, 384], F32, tag="ps")
            for c in range(HCH):
                nc.tensor.matmul(ps_gx[:], xc_sb[:, c:c + 1], wih[:, c, :],
                                 start=(c == 0), stop=(c == 7))
            if with_bias:
                t1 = sb.tile([1, 384], F32)
                nc.vector.tensor_tensor(t1[:], ps_gx[:], rows[:, 1216:1600],
                                        op=OP.add)
                t2 = sb.tile([1, 384], F32)
                nc.vector.tensor_tensor(t2[:], ps_gh[:], rows[:, 1600:1984],
                                        op=OP.add)
            else:
                # DVE reads at most one PSUM operand; gh is ready early,
                # so its copy to SBUF overlaps the combine phase.
                t1 = ps_gx
                t2 = sb.tile([1, 384], F32)
                nc.scalar.copy(t2[:], ps_gh[:])
            rz = sb.tile([1, 256], F32)
            nc.vector.tensor_tensor(rz[:], t1[:, 0:256], t2[:, 0:256], op=OP.add)
            # sigmoid(x) = 0.5 + 0.5 * tanh(x/2)  (tanh shares the exp table set)
            rzt = sb.tile([1, 256], F32)
            nc.scalar.activation(rzt[:], rz[:], AF.Tanh, scale=0.5)
            rzs = sb.tile([1, 256], F32)
            nc.vector.tensor_scalar(rzs[:], rzt[:], 0.5, 0.5,
                                    op0=OP.mult, op1=OP.add)
            npre = sb.tile([1, 128], F32)
            nc.vector.tensor_tensor(npre[:], rzs[:, 0:128], t2[:, 256:384],
                                    op=OP.mult)
            npre2 = sb.tile([1, 128], F32)
            nc.vector.tensor_tensor(npre2[:], npre[:], t1[:, 256:384], op=OP.add)
            nt = sb.tile([1, 128], F32)
            nc.scalar.activation(nt[:], npre2[:], AF.Tanh)
            dd = sb.tile([1, 128], F32)
            nc.vector.tensor_tensor(dd[:], rows[:, 64:192], nt[:], op=OP.subtract)
            zd = sb.tile([1, 128], F32)
            nc.vector.tensor_tensor(zd[:], rzs[:, 128:256], dd[:], op=OP.mult)
            hnew_sb = sb.tile([1, 128], F32)
            nc.vector.tensor_tensor(hnew_sb[:], nt[:], zd[:], op=OP.add)

            # ---- AllGather h_new ----
            cc_h_in = dram.tile([1, 128], F32, tag="cchi")
            cc_h_out = dram.tile([N_CORES, 128], F32, tag="ccho")
            dma_cchi = nc.gpsimd.dma_start(cc_h_in[:], hnew_sb[:])
            for t in range(5, 10):
                tile.add_dep_helper(wo_dmas[t].ins, dma_cchi.ins, sync=True,
                                    reason="wout mid tiles fill the AG-h window")
            nc.gpsimd.collective_compute(
                "AllGather", OP.bypass,
                replica_groups=[list(range(N_CORES))],
                ins=[cc_h_in[:]], outs=[cc_h_out[:]],
            )
            gh2_sb = sb.tile([N_CORES, 128], F32)
            dma_gh2 = nc.gpsimd.dma_start(gh2_sb[:], cc_h_out[:])
            for t in range(10, VT):
                tile.add_dep_helper(wo_dmas[t].ins, dma_gh2.ins, sync=True,
                                    reason="wout tail tiles after last chain DMA")
            ps_hc = ps.tile([128, HCH], F32, tag="ps")
            nc.tensor.matmul(ps_hc[:], gh2_sb[:], eye[0:8, 0:8],
                             start=True, stop=True)
            hc_sb = sb.tile([128, HCH], BF16)
            nc.scalar.copy(hc_sb[:], ps_hc[:])

            # ---- logits: 13 x [1, 512] matvecs against resident W_out ----
            logits_sb = sb.tile([VT, TN], F32)
            sev = sb.tile([1, 16], F32)
            for t in range(VT):
                ps_t = psL.tile([1, TN], F32, tag="pst")
                if with_bias:
                    # fold b_out[t] in via a one-hot selector matmul
                    nc.tensor.matmul(ps_t[:], eyeb[0:VT, t:t + 1], boutb[:],
                                     start=True, stop=False)
                for c in range(HCH):
                    nc.tensor.matmul(ps_t[:], hc_sb[:, c:c + 1],
                                     wo_tiles[t][:, c, :],
                                     start=(not with_bias and c == 0),
                                     stop=(c == 7))
                row_t = rowp.tile([1, TN], F32, tag="row")
                nc.scalar.copy(row_t[:], ps_t[:])
                # max-free softmax: |logits| << 88, exp cannot overflow fp32
                exp_t = rowp.tile([1, TN], F32, tag="erow")
                nc.scalar.activation(exp_t[:], row_t[:], AF.Exp)
                nc.vector.reduce_sum(sev[0:1, t:t + 1], exp_t[:], axis=AX.X)
                # partition shift via SBUF->SBUF DMA
                nc.scalar.dma_start(logits_sb[t:t + 1, :], row_t[:])

            ms_sb = sb.tile([1, 1], F32)
            if with_bias:
                nc.vector.reduce_sum(ms_sb[:], sev[0:1, 0:VT], axis=AX.X)
            else:
                # zero-weight pad rows contribute exp(0)=1 each; remove them
                msr = sb.tile([1, 1], F32)
                nc.vector.reduce_sum(msr[:], sev[0:1, 0:VT], axis=AX.X)
                nc.vector.tensor_tensor(ms_sb[:], msr[:], padc[:],
                                        op=OP.subtract)

            # ---- AllGather s_k ----
            cc_ms_in = dram.tile([1, 1], F32, tag="ccmi")
            cc_ms_out = dram.tile([N_CORES, 1], F32, tag="ccmo")
            nc.gpsimd.dma_start(cc_ms_in[:], ms_sb[:])
            nc.gpsimd.collective_compute(
                "AllGather", OP.bypass,
                replica_groups=[list(range(N_CORES))],
                ins=[cc_ms_in[:]], outs=[cc_ms_out[:]],
            )
            msg_sb = sb.tile([1, N_CORES], F32)
            nc.gpsimd.dma_start(msg_sb[:],
                                cc_ms_out[:].rearrange("(o a) b -> o (a b)", o=1))
            Sg = sb.tile([1, 1], F32)
            nc.vector.reduce_sum(Sg[:], msg_sb[:], axis=AX.X)
            logS = sb.tile([1, 1], F32)
            nc.scalar.activation(logS[:], Sg[:], AF.Ln)
            ps_cb = ps.tile([VT, 1], F32, tag="ps")
            nc.tensor.matmul(ps_cb[:], onesr[0:1, 0:VT], logS[:],
                             start=True, stop=True)
            ccol = sb.tile([VT, 1], F32)
            nc.scalar.copy(ccol[:], ps_cb[:])
            nc.sync.dma_start(attn_out[:], attnw_sb[:])
            nc.sync.dma_start(hnew_out[:], hnew_sb[:])
            logp_sb = sb.tile([VT, TN], F32)
            nc.vector.tensor_scalar_sub(logp_sb[:], logits_sb[:], ccol[:])
            nc.scalar.dma_start(logp_out[:], logp_sb[:])

    nc.compile()
    _NC_CACHE[key] = nc
    return nc


def _prep_inputs(inp, hidden, encoder_outputs, emb, W_attn, b_attn, W_comb,
                 b_comb, W_ih, W_hh, b_ih, b_hh, W_out, b_out):
    f = np.float32
    idx = int(np.asarray(inp).reshape(-1)[0])
    e = np.ascontiguousarray(np.asarray(emb)[idx], dtype=f).reshape(H)
    hv = np.ascontiguousarray(np.asarray(hidden), dtype=f).reshape(H)
    enc = np.ascontiguousarray(np.asarray(encoder_outputs), dtype=f)
    W_attn = np.asarray(W_attn, dtype=f)
    b_attn = np.asarray(b_attn, dtype=f)
    W_comb = np.asarray(W_comb, dtype=f)
    b_comb = np.asarray(b_comb, dtype=f)
    W_ih = np.asarray(W_ih, dtype=f)
    W_hh = np.asarray(W_hh, dtype=f)
    b_ih = np.asarray(b_ih, dtype=f)
    b_hh = np.asarray(b_hh, dtype=f)
    W_out = np.asarray(W_out, dtype=f)
    b_out = np.asarray(b_out, dtype=f)

    cols = np.empty((128, 17), dtype=f)
    cols[:, 0:8] = e.reshape(8, 128).T
    cols[:, 8:16] = hv.reshape(8, 128).T
    cols[:, 16] = 1.0

    # packA (f32r): colsr | wattnT[p, c*64+j] = W_attn[j, c*128+p]
    packA = np.empty((128, PA_W), dtype=f)
    packA[:, 0:17] = cols
    packA[:, 17:] = (W_attn.reshape(L, 16, 128).transpose(2, 1, 0)
                     .reshape(128, 16 * L))

    # packD (f32, 16 partitions): rows | eye | ones | bout_row
    packD_base = np.zeros((16, PD_W), dtype=f)
    packD_base[0:16, PD_EYE:PD_EYE + 16] = np.eye(16, dtype=f)
    packD_base[0:1, PD_ONESR:PD_ONESR + 16] = 1.0

    W_comb_b = W_comb.astype(BF)
    W_ih_b = W_ih.astype(BF)
    W_hh_b = W_hh.astype(BF)
    W_out_b = W_out.astype(BF)
    cols_b = cols.astype(BF)

    # packB (bf16): colsb | encb | wcombT e-half; packB2: app-half (replicated)
    wcombT_full = (W_comb_b.reshape(H, 16, 128).transpose(2, 1, 0)
                   .reshape(128, 16, H))
    packB_base = np.zeros((128, PB1_W), dtype=BF)
    packB_base[:, 0:17] = cols_b
    packB_base[0:16, PB_OFF_EYE:PB_OFF_EYE + 16] = np.eye(16, dtype=np.float32)
    packB_base[0:64, PB_OFF_ENC:PB_OFF_ENC + H] = enc.astype(BF)
    packB_base[:, PB_OFF_WC:] = wcombT_full[:, 0:HCH].reshape(128, HCH * H)
    packB2 = np.ascontiguousarray(wcombT_full[:, HCH:16].reshape(128, HCH * H))

    in_maps = []
    for k in range(N_CORES):
        # packC (bf16): wihT | whhT, [p, c*384 + g*128 + j]
        packC = np.empty((128, PC_W), dtype=BF)
        packC[:, 0:HCH * 384] = (
            W_ih_b.reshape(3, 8, 128, H)[:, k].reshape(3, 128, 8, 128)
            .transpose(3, 2, 0, 1).reshape(128, HCH * 384))
        packC[:, HCH * 384:] = (
            W_hh_b.reshape(3, 8, 128, H)[:, k].reshape(3, 128, 8, 128)
            .transpose(3, 2, 0, 1).reshape(128, HCH * 384))

        packD = packD_base.copy()
        packD[0:1, PD_ROWS:PD_ROWS + 64] = b_attn
        packD[0:1, PD_ROWS + 64:PD_ROWS + 192] = hv[k * 128:(k + 1) * 128]
        packD[0:1, PD_ROWS + 192:PD_ROWS + 1216] = b_comb
        packD[0:1, PD_ROWS + 1216:PD_ROWS + 1600] = (
            b_ih.reshape(3, 8, 128)[:, k, :].reshape(384))
        packD[0:1, PD_ROWS + 1600:PD_ROWS + 1984] = (
            b_hh.reshape(3, 8, 128)[:, k, :].reshape(384))

        # vocab shard (pad to VPC rows)
        v0 = k * VPC
        v1 = min((k + 1) * VPC, V)
        nreal = max(0, v1 - v0)
        packD[0, PD_PADC] = float(VPC - nreal)
        Wk = np.zeros((VPC, H), dtype=BF)
        bk = np.full((VPC,), NEG_BIG, dtype=f)
        if nreal > 0:
            Wk[:nreal] = W_out_b[v0:v1]
            bk[:nreal] = b_out[v0:v1]
        packB = packB_base.copy()
        packB[0:VT, PB_OFF_BOUT:PB_OFF_BOUT + TN] = bk.reshape(VT, TN).astype(BF)
        # woutT[t, p, c, n] = Wk[t*512 + n, c*128 + p]
        woutT = np.ascontiguousarray(
            Wk.reshape(VT, TN, 8, 128).transpose(0, 3, 2, 1))

        in_maps.append({
            "packA": packA,
            "packB": packB,
            "packB2": packB2,
            "packC": packC,
            "packD": packD,
            "woutT": woutT,
        })
    return in_maps


def kernel(**inputs):
    global LAST_EXEC_NS, LAST_TRACE_PATH
    with_bias = any(
        np.any(np.asarray(inputs[n]))
        for n in ("b_attn", "b_comb", "b_ih", "b_hh", "b_out"))
    nc = _build_nc(with_bias=with_bias)
    in_maps = _prep_inputs(**inputs)
    kwargs = {}
    if TRACE:
        kwargs = dict(trace=True)
    res = run_bass_kernel_spmd(nc, in_maps, core_ids=list(range(N_CORES)),
                               **kwargs)
    LAST_EXEC_NS = res.exec_time_ns
    if res.instructions_and_trace is not None:
        LAST_TRACE_PATH = res.instructions_and_trace[1]

    logp = np.concatenate(
        [res.results[k]["logp"].reshape(-1) for k in range(N_CORES)])[:V]
    logp = logp.reshape(1, V).astype(np.float32)
    h_new = np.concatenate(
        [res.results[k]["hnew"].reshape(-1) for k in range(N_CORES)])
    h_new = h_new.reshape(1, 1, H).astype(np.float32)
    attn_w = res.results[0]["attnw"].reshape(1, L).astype(np.float32)
    return (logp, h_new, attn_w)


# revision 27
# speedup vs baseline: 1.0372x; 1.0372x over previous
"""AttentionDecoderGRU single-step kernel for 8 Trainium2 NeuronCores.

Strategy (batch=1 matrix-vector workload, HBM-bandwidth bound):
  - Shard every weight matrix across the 8 cores:
      W_attn/W_comb/W_ih/W_hh by output dim (each core computes a 128-slice),
      W_out/b_out by vocab dim (6656 padded rows per core).
  - Embedding lookup (one row of emb) happens on the host (it is pure input
    sharding: only the selected row is shipped).
  - All matvecs run on the TensorEngine with host-pre-transposed weights.
    W_comb/W_ih/W_hh/W_out are bf16 (halves HBM traffic; rel err ~1e-3,
    threshold 2e-2); the attention path stays fp32r/fp32.
  - Small inputs are packed into 4 tensors so the serial chain isn't
    gated on a dozen individual DMA fixed costs.
  - 3 small AllGather collectives stitch the cores together: x (combine
    output), h_new, and the per-core (max, sumexp) pairs for log_softmax.
  - DMA rings are split: the 13 MB/core W_out stream goes through nc.sync
    (SP HWDGE ring) and is fully SBUF-resident before use; everything
    latency-critical goes through nc.scalar (ACT ring); collectives
    through gpsimd.

Outputs per core: logp shard [13, 512], h_new shard [1, 128], attn_w [1, 64].
The host concatenates shards and trims vocab padding.
"""

import ml_dtypes
import numpy as np

import concourse.bacc as bacc
import concourse.mybir as mybir
import concourse.tile as tile
from concourse.bass_utils import run_bass_kernel_spmd

N_CORES = 8
H = 1024
L = 64
V = 50257
HCH = 8          # H / 128 chunks
VPC = 6656       # padded vocab rows per core (= VT * TN)
VT = 13
TN = 512
NEG_BIG = -1.0e30

F32 = mybir.dt.float32
F32R = mybir.dt.float32r
BF16 = mybir.dt.bfloat16
AF = mybir.ActivationFunctionType
AX = mybir.AxisListType
OP = mybir.AluOpType

BF = ml_dtypes.bfloat16

# pack layouts (columns)
PA_W = 17 + 16 * L                # f32r: colsr | wattnT
PB_OFF_EYE = 17
PB_OFF_BOUT = 33
PB_OFF_ENC = 545
PB_OFF_WC = PB_OFF_ENC + H
PB1_W = PB_OFF_WC + 8 * H        # bf16: colsb | encb | wcombT[c=0..7]
PB2_W = 8 * H                    # bf16: wcombT[c=8..15]
PC_W = 2 * HCH * 384              # bf16: wihT | whhT
# packD (f32, 16 partitions): rows | eye16 | ones_row | bout_row
PD_ROWS = 0
PD_EYE = 1984
PD_ONESR = PD_EYE + 16
PD_PADC = PD_ONESR + 16
PD_W = PD_PADC + 1

TRACE = False
LAST_EXEC_NS = None
LAST_TRACE_PATH = None

_NC_CACHE = {}


def _build_nc(with_bias=True):
    key = ("nc", with_bias)
    if key in _NC_CACHE:
        return _NC_CACHE[key]

    nc = bacc.Bacc("TRN2", target_bir_lowering=False, debug=False,
                   num_devices=N_CORES)

    # ---- I/O ----
    packA_dr = nc.dram_tensor("packA", [128, PA_W], F32R, kind="ExternalInput")
    packB_dr = nc.dram_tensor("packB", [128, PB1_W], BF16, kind="ExternalInput")
    packB2_dr = nc.dram_tensor("packB2", [128, PB2_W], BF16, kind="ExternalInput")
    packC_dr = nc.dram_tensor("packC", [128, PC_W], BF16, kind="ExternalInput")
    packD_dr = nc.dram_tensor("packD", [16, PD_W], F32, kind="ExternalInput")
    wout_dr = nc.dram_tensor("woutT", [VT, 128, HCH, TN], BF16,
                             kind="ExternalInput")

    logp_out = nc.dram_tensor("logp", [VT, TN], F32, kind="ExternalOutput")
    hnew_out = nc.dram_tensor("hnew", [1, 128], F32, kind="ExternalOutput")
    attn_out = nc.dram_tensor("attnw", [1, L], F32, kind="ExternalOutput")

    with tile.TileContext(nc) as tc:
        with (
            tc.tile_pool(name="sb", bufs=1) as sb,
            tc.tile_pool(name="wpool", bufs=VT) as wpool,
            tc.tile_pool(name="rowp", bufs=3) as rowp,
            tc.tile_pool(name="ps", bufs=4, space="PSUM") as ps,
            tc.tile_pool(name="psL", bufs=3, space="PSUM") as psL,
            tc.tile_pool(name="dram", bufs=1, space="DRAM") as dram,
        ):
            # ---- packed small loads (ACT ring), in dependency order ----
            pA = sb.tile([128, PA_W], F32R)
            dma_pA = nc.scalar.dma_start(pA[:], packA_dr[:])
            pD = sb.tile([16, PD_W], F32)
            dma_pD = nc.scalar.dma_start(pD[:], packD_dr[:])
            pB = sb.tile([128, PB1_W], BF16)
            dma_pB = nc.scalar.dma_start(pB[:], packB_dr[:])
            pC = sb.tile([128, PC_W], BF16)
            dma_pC = nc.scalar.dma_start(pC[:], packC_dr[:])
            pB2 = sb.tile([128, PB2_W], BF16)
            dma_pB2 = nc.scalar.dma_start(pB2[:], packB2_dr[:])

            colsr = pA[:, 0:17]
            wattn = pA[:, 17:PA_W].rearrange("p (c n) -> p c n", c=16)
            colsb = pB[:, 0:17]
            eyeb = pB[0:16, PB_OFF_EYE:PB_OFF_EYE + 16]
            boutb = pB[0:VT, PB_OFF_BOUT:PB_OFF_BOUT + TN]
            encb = pB[0:64, PB_OFF_ENC:PB_OFF_ENC + H]
            wcomb = pB[:, PB_OFF_WC:PB1_W].rearrange("p (c n) -> p c n", c=HCH)
            wcomb2 = pB2[:].rearrange("p (c n) -> p c n", c=HCH)
            wih = pC[:, 0:HCH * 384].rearrange("p (c n) -> p c n", c=HCH)
            whh = pC[:, HCH * 384:PC_W].rearrange("p (c n) -> p c n", c=HCH)
            rows = pD[0:1, PD_ROWS:PD_ROWS + 1984]
            eye = pD[0:16, PD_EYE:PD_EYE + 16]
            onesr = pD[0:1, PD_ONESR:PD_ONESR + 16]
            padc = pD[0:1, PD_PADC:PD_PADC + 1]

            # W_out shard: fully SBUF-resident, 13 tile DMAs (SP ring),
            # staged behind chain events so the serial DMA pipe always
            # serves the latency-critical chain first.
            wo_tiles = []
            wo_dmas = []
            for t in range(VT):
                wo = wpool.tile([128, HCH, TN], BF16, tag="wo")
                dma_wo = nc.sync.dma_start(wo[:], wout_dr[t])
                wo_dmas.append(dma_wo)
                if t < 5:
                    tile.add_dep_helper(dma_wo.ins, dma_pB2.ins, sync=True,
                                        reason="wout after critical packs")
                wo_tiles.append(wo)

            # ---- attention scores: s = [e,h] @ W_attn.T + b_attn ----
            ps_s = ps.tile([1, L], F32, tag="ps")
            for c in range(16):
                nc.tensor.matmul(ps_s[:], colsr[:, c:c + 1], wattn[:, c, :],
                                 start=(c == 0), stop=(c == 15))
            es = sb.tile([1, L], F32)
            if with_bias:
                s_sb = sb.tile([1, L], F32)
                nc.vector.tensor_tensor(s_sb[:], ps_s[:], rows[:, 0:L],
                                        op=OP.add)
                nc.scalar.activation(es[:], s_sb[:], AF.Exp)
            else:
                nc.scalar.activation(es[:], ps_s[:], AF.Exp)
            ssum = sb.tile([1, 1], F32)
            nc.vector.reduce_sum(ssum[:], es[:], axis=AX.X)
            rcp = sb.tile([1, 1], F32)
            nc.vector.reciprocal(rcp[:], ssum[:])
            attnw_sb = sb.tile([1, L], F32)
            nc.vector.tensor_scalar_mul(attnw_sb[:], es[:], rcp[:])

            # attn_w row -> column [64, 1]
            ps_awc = ps.tile([L, 1], F32, tag="ps")
            nc.tensor.matmul(ps_awc[:], attnw_sb[:], onesr[0:1, 0:1],
                             start=True, stop=True)
            awc_sb = sb.tile([L, 1], BF16)
            nc.scalar.copy(awc_sb[:], ps_awc[:])

            # applied = attn_w @ enc, directly as bf16 column chunks [128, 8]
            ps_app = ps.tile([128, HCH], F32, tag="ps")
            for c in range(HCH):
                nc.tensor.matmul(ps_app[:, c:c + 1],
                                 encb[:, c * 128:(c + 1) * 128], awc_sb[:],
                                 start=True, stop=True)
            appc_sb = sb.tile([128, HCH], BF16)
            nc.scalar.copy(appc_sb[:], ps_app[:])

            ps_gh = ps.tile([1, 384], F32, tag="ps")
            for c in range(HCH):
                nc.tensor.matmul(ps_gh[:], colsb[:, 8 + c:9 + c], whh[:, c, :],
                                 start=(c == 0), stop=(c == 7))
            # ---- combine (replicated): x = relu([e, applied] @ W_comb.T + b) ----
            ps_x1 = ps.tile([1, 512], F32, tag="ps")
            ps_x2 = ps.tile([1, 512], F32, tag="ps")
            for c in range(HCH):
                nc.tensor.matmul(ps_x1[:], colsb[:, c:c + 1],
                                 wcomb[:, c, 0:512], start=(c == 0), stop=False)
                nc.tensor.matmul(ps_x2[:], colsb[:, c:c + 1],
                                 wcomb[:, c, 512:1024], start=(c == 0), stop=False)
            for c in range(HCH):
                nc.tensor.matmul(ps_x1[:], appc_sb[:, c:c + 1],
                                 wcomb2[:, c, 0:512], start=False, stop=(c == 7))
                nc.tensor.matmul(ps_x2[:], appc_sb[:, c:c + 1],
                                 wcomb2[:, c, 512:1024], start=False, stop=(c == 7))
            x_sb = sb.tile([1, H], F32)
            if with_bias:
                xr = sb.tile([1, H], F32)
                nc.vector.tensor_tensor(xr[:, 0:512], ps_x1[:],
                                        rows[:, 192:704], op=OP.add)
                nc.vector.tensor_tensor(xr[:, 512:1024], ps_x2[:],
                                        rows[:, 704:1216], op=OP.add)
                nc.vector.tensor_scalar_max(x_sb[:], xr[:], 0.0)
            else:
                nc.vector.tensor_scalar_max(x_sb[:, 0:512], ps_x1[:], 0.0)
                nc.vector.tensor_scalar_max(x_sb[:, 512:1024], ps_x2[:], 0.0)
            # x row -> bf16 column chunks for the gx matvecs
            ps_xc = ps.tile([128, HCH], F32, tag="ps")
            for c in range(HCH):
                nc.tensor.matmul(ps_xc[:, c:c + 1],
                                 x_sb[0:1, c * 128:(c + 1) * 128],
                                 onesr[0:1, 0:1], start=True, stop=True)
            xc_sb = sb.tile([128, HCH], BF16)
            nc.scalar.copy(xc_sb[:], ps_xc[:])

            # ---- GRU gates ----
            ps_gx = ps.tile([1, 384], F32, tag="ps")
            for c in range(HCH):
                nc.tensor.matmul(ps_gx[:], xc_sb[:, c:c + 1], wih[:, c, :],
                                 start=(c == 0), stop=(c == 7))
            if with_bias:
                t1 = sb.tile([1, 384], F32)
                nc.vector.tensor_tensor(t1[:], ps_gx[:], rows[:, 1216:1600],
                                        op=OP.add)
                t2 = sb.tile([1, 384], F32)
                nc.vector.tensor_tensor(t2[:], ps_gh[:], rows[:, 1600:1984],
                                        op=OP.add)
            else:
                # DVE reads at most one PSUM operand; gh is ready early,
                # so its copy to SBUF overlaps the combine phase.
                t1 = ps_gx
                t2 = sb.tile([1, 384], F32)
                nc.scalar.copy(t2[:], ps_gh[:])
            rz = sb.tile([1, 256], F32)
            nc.vector.tensor_tensor(rz[:], t1[:, 0:256], t2[:, 0:256], op=OP.add)
            # sigmoid(x) = 0.5 + 0.5 * tanh(x/2)  (tanh shares the exp table set)
            rzt = sb.tile([1, 256], F32)
            nc.scalar.activation(rzt[:], rz[:], AF.Tanh, scale=0.5)
            rzs = sb.tile([1, 256], F32)
            nc.vector.tensor_scalar(rzs[:], rzt[:], 0.5, 0.5,
                                    op0=OP.mult, op1=OP.add)
            npre = sb.tile([1, 128], F32)
            nc.vector.tensor_tensor(npre[:], rzs[:, 0:128], t2[:, 256:384],
                                    op=OP.mult)
            npre2 = sb.tile([1, 128], F32)
            nc.vector.tensor_tensor(npre2[:], npre[:], t1[:, 256:384], op=OP.add)
            nt = sb.tile([1, 128], F32)
            nc.scalar.activation(nt[:], npre2[:], AF.Tanh)
            dd = sb.tile([1, 128], F32)
            nc.vector.tensor_tensor(dd[:], rows[:, 64:192], nt[:], op=OP.subtract)
            zd = sb.tile([1, 128], F32)
            nc.vector.tensor_tensor(zd[:], rzs[:, 128:256], dd[:], op=OP.mult)
            hnew_sb = sb.tile([1, 128], F32)
            nc.vector.tensor_tensor(hnew_sb[:], nt[:], zd[:], op=OP.add)

            # ---- AllGather h_new ----
            cc_h_in = dram.tile([1, 128], F32, tag="cchi")
            cc_h_out = dram.tile([N_CORES, 128], F32, tag="ccho")
            dma_cchi = nc.gpsimd.dma_start(cc_h_in[:], hnew_sb[:])
            for t in range(5, 10):
                tile.add_dep_helper(wo_dmas[t].ins, dma_cchi.ins, sync=True,
                                    reason="wout mid tiles fill the AG-h window")
            nc.gpsimd.collective_compute(
                "AllGather", OP.bypass,
                replica_groups=[list(range(N_CORES))],
                ins=[cc_h_in[:]], outs=[cc_h_out[:]],
            )
            gh2_sb = sb.tile([N_CORES, 128], F32)
            dma_gh2 = nc.gpsimd.dma_start(gh2_sb[:], cc_h_out[:])
            for t in range(10, VT):
                tile.add_dep_helper(wo_dmas[t].ins, dma_gh2.ins, sync=True,
                                    reason="wout tail tiles after last chain DMA")
            ps_hc = ps.tile([128, HCH], F32, tag="ps")
            nc.tensor.matmul(ps_hc[:], gh2_sb[:], eye[0:8, 0:8],
                             start=True, stop=True)
            hc_sb = sb.tile([128, HCH], BF16)
            nc.scalar.copy(hc_sb[:], ps_hc[:])

            # ---- logits: 13 x [1, 512] matvecs against resident W_out ----
            logits_sb = sb.tile([VT, TN], F32)
            sev = sb.tile([1, 16], F32)
            for t in range(VT):
                ps_t = psL.tile([1, TN], F32, tag="pst")
                if with_bias:
                    # fold b_out[t] in via a one-hot selector matmul
                    nc.tensor.matmul(ps_t[:], eyeb[0:VT, t:t + 1], boutb[:],
                                     start=True, stop=False)
                for c in range(HCH):
                    nc.tensor.matmul(ps_t[:], hc_sb[:, c:c + 1],
                                     wo_tiles[t][:, c, :],
                                     start=(not with_bias and c == 0),
                                     stop=(c == 7))
                row_t = rowp.tile([1, TN], F32, tag="row")
                nc.scalar.copy(row_t[:], ps_t[:])
                # max-free softmax: |logits| << 88, exp cannot overflow fp32
                exp_t = rowp.tile([1, TN], F32, tag="erow")
                nc.scalar.activation(exp_t[:], row_t[:], AF.Exp)
                nc.vector.reduce_sum(sev[0:1, t:t + 1], exp_t[:], axis=AX.X)
                # partition shift via SBUF->SBUF DMA
                nc.scalar.dma_start(logits_sb[t:t + 1, :], row_t[:])

            ms_sb = sb.tile([1, 1], F32)
            if with_bias:
                nc.vector.reduce_sum(ms_sb[:], sev[0:1, 0:VT], axis=AX.X)
            else:
                # zero-weight pad rows contribute exp(0)=1 each; remove them
                msr = sb.tile([1, 1], F32)
                nc.vector.reduce_sum(msr[:], sev[0:1, 0:VT], axis=AX.X)
                nc.vector.tensor_tensor(ms_sb[:], msr[:], padc[:],
                                        op=OP.subtract)

            # ---- AllGather s_k ----
            cc_ms_in = dram.tile([1, 1], F32, tag="ccmi")
            cc_ms_out = dram.tile([N_CORES, 1], F32, tag="ccmo")
            nc.gpsimd.dma_start(cc_ms_in[:], ms_sb[:])
            nc.gpsimd.collective_compute(
                "AllGather", OP.bypass,
                replica_groups=[list(range(N_CORES))],
                ins=[cc_ms_in[:]], outs=[cc_ms_out[:]],
            )
            msg_sb = sb.tile([1, N_CORES], F32)
            nc.gpsimd.dma_start(msg_sb[:],
                                cc_ms_out[:].rearrange("(o a) b -> o (a b)", o=1))
            Sg = sb.tile([1, 1], F32)
            nc.vector.reduce_sum(Sg[:], msg_sb[:], axis=AX.X)
            logS = sb.tile([1, 1], F32)
            nc.scalar.activation(logS[:], Sg[:], AF.Ln)
            ps_cb = ps.tile([VT, 1], F32, tag="ps")
            nc.tensor.matmul(ps_cb[:], onesr[0:1, 0:VT], logS[:],
                             start=True, stop=True)
            ccol = sb.tile([VT, 1], F32)
            nc.scalar.copy(ccol[:], ps_cb[:])
            nc.sync.dma_start(attn_out[:], attnw_sb[:])
            nc.sync.dma_start(hnew_out[:], hnew_sb[:])
            logp_sb = sb.tile([VT, TN], F32)
            nc.vector.tensor_scalar_sub(logp_sb[:], logits_sb[:], ccol[:])
            nc.scalar.dma_start(logp_out[:], logp_sb[:])

    nc.compile()
    _NC_CACHE[key] = nc
    return nc


def _prep_inputs(inp, hidden, encoder_outputs, emb, W_attn, b_attn, W_comb,
                 b_comb, W_ih, W_hh, b_ih, b_hh, W_out, b_out):
    f = np.float32
    idx = int(np.asarray(inp).reshape(-1)[0])
    e = np.ascontiguousarray(np.asarray(emb)[idx], dtype=f).reshape(H)
    hv = np.ascontiguousarray(np.asarray(hidden), dtype=f).reshape(H)
    enc = np.ascontiguousarray(np.asarray(encoder_outputs), dtype=f)
    W_attn = np.asarray(W_attn, dtype=f)
    b_attn = np.asarray(b_attn, dtype=f)
    W_comb = np.asarray(W_comb, dtype=f)
    b_comb = np.asarray(b_comb, dtype=f)
    W_ih = np.asarray(W_ih, dtype=f)
    W_hh = np.asarray(W_hh, dtype=f)
    b_ih = np.asarray(b_ih, dtype=f)
    b_hh = np.asarray(b_hh, dtype=f)
    W_out = np.asarray(W_out, dtype=f)
    b_out = np.asarray(b_out, dtype=f)

    cols = np.empty((128, 17), dtype=f)
    cols[:, 0:8] = e.reshape(8, 128).T
    cols[:, 8:16] = hv.reshape(8, 128).T
    cols[:, 16] = 1.0

    # packA (f32r): colsr | wattnT[p, c*64+j] = W_attn[j, c*128+p]
    packA = np.empty((128, PA_W), dtype=f)
    packA[:, 0:17] = cols
    packA[:, 17:] = (W_attn.reshape(L, 16, 128).transpose(2, 1, 0)
                     .reshape(128, 16 * L))

    # packD (f32, 16 partitions): rows | eye | ones | bout_row
    packD_base = np.zeros((16, PD_W), dtype=f)
    packD_base[0:16, PD_EYE:PD_EYE + 16] = np.eye(16, dtype=f)
    packD_base[0:1, PD_ONESR:PD_ONESR + 16] = 1.0

    W_comb_b = W_comb.astype(BF)
    W_ih_b = W_ih.astype(BF)
    W_hh_b = W_hh.astype(BF)
    W_out_b = W_out.astype(BF)
    cols_b = cols.astype(BF)

    # packB (bf16): colsb | encb | wcombT e-half; packB2: app-half (replicated)
    wcombT_full = (W_comb_b.reshape(H, 16, 128).transpose(2, 1, 0)
                   .reshape(128, 16, H))
    packB_base = np.zeros((128, PB1_W), dtype=BF)
    packB_base[:, 0:17] = cols_b
    packB_base[0:16, PB_OFF_EYE:PB_OFF_EYE + 16] = np.eye(16, dtype=np.float32)
    packB_base[0:64, PB_OFF_ENC:PB_OFF_ENC + H] = enc.astype(BF)
    packB_base[:, PB_OFF_WC:] = wcombT_full[:, 0:HCH].reshape(128, HCH * H)
    packB2 = np.ascontiguousarray(wcombT_full[:, HCH:16].reshape(128, HCH * H))

    in_maps = []
    for k in range(N_CORES):
        # packC (bf16): wihT | whhT, [p, c*384 + g*128 + j]
        packC = np.empty((128, PC_W), dtype=BF)
        packC[:, 0:HCH * 384] = (
            W_ih_b.reshape(3, 8, 128, H)[:, k].reshape(3, 128, 8, 128)
            .transpose(3, 2, 0, 1).reshape(128, HCH * 384))
        packC[:, HCH * 384:] = (
            W_hh_b.reshape(3, 8, 128, H)[:, k].reshape(3, 128, 8, 128)
            .transpose(3, 2, 0, 1).reshape(128, HCH * 384))

        packD = packD_base.copy()
        packD[0:1, PD_ROWS:PD_ROWS + 64] = b_attn
        packD[0:1, PD_ROWS + 64:PD_ROWS + 192] = hv[k * 128:(k + 1) * 128]
        packD[0:1, PD_ROWS + 192:PD_ROWS + 1216] = b_comb
        packD[0:1, PD_ROWS + 1216:PD_ROWS + 1600] = (
            b_ih.reshape(3, 8, 128)[:, k, :].reshape(384))
        packD[0:1, PD_ROWS + 1600:PD_ROWS + 1984] = (
            b_hh.reshape(3, 8, 128)[:, k, :].reshape(384))

        # vocab shard (pad to VPC rows)
        v0 = k * VPC
        v1 = min((k + 1) * VPC, V)
        nreal = max(0, v1 - v0)
        packD[0, PD_PADC] = float(VPC - nreal)
        Wk = np.zeros((VPC, H), dtype=BF)
        bk = np.full((VPC,), NEG_BIG, dtype=f)
        if nreal > 0:
            Wk[:nreal] = W_out_b[v0:v1]
            bk[:nreal] = b_out[v0:v1]
        packB = packB_base.copy()
        packB[0:VT, PB_OFF_BOUT:PB_OFF_BOUT + TN] = bk.reshape(VT, TN).astype(BF)
        # woutT[t, p, c, n] = Wk[t*512 + n, c*128 + p]
        woutT = np.ascontiguousarray(
            Wk.reshape(VT, TN, 8, 128).transpose(0, 3, 2, 1))

        in_maps.append({
            "packA": packA,
            "packB": packB,
            "packB2": packB2,
            "packC": packC,
            "packD": packD,
            "woutT": woutT,
        })
    return in_maps


def kernel(**inputs):
    global LAST_EXEC_NS, LAST_TRACE_PATH
    # The bias-free variant measured no faster (the adds sit off the
    # modeled critical path), so always use the general with-bias program.
    nc = _build_nc(with_bias=True)
    in_maps = _prep_inputs(**inputs)
    kwargs = {}
    if TRACE:
        kwargs = dict(trace=True)
    res = run_bass_kernel_spmd(nc, in_maps, core_ids=list(range(N_CORES)),
                               **kwargs)
    LAST_EXEC_NS = res.exec_time_ns
    if res.instructions_and_trace is not None:
        LAST_TRACE_PATH = res.instructions_and_trace[1]

    logp = np.concatenate(
        [res.results[k]["logp"].reshape(-1) for k in range(N_CORES)])[:V]
    logp = logp.reshape(1, V).astype(np.float32)
    h_new = np.concatenate(
        [res.results[k]["hnew"].reshape(-1) for k in range(N_CORES)])
    h_new = h_new.reshape(1, 1, H).astype(np.float32)
    attn_w = res.results[0]["attnw"].reshape(1, L).astype(np.float32)
    return (logp, h_new, attn_w)
